# revision 21
# baseline (speedup 1.0000x reference)
"""Trainium2 Bass kernel for nn_CBF (dense MLP forward + Jacobian row).

Math: HJH = [h, Jh] with h = MLP(x_norm) (scalar) and
  Jh = Wout @ W3 @ D2 @ W2 @ D1 @ W1 @ D0 @ (W0 / x_range)  (1 x n row).
The Jacobian chain collapses to r0 @ W0 / x_range with r0 a 128-vector,
so only two passes over the big W0 (128 x 131072) are needed:
  pass 1: V0 = x_norm @ W0.T   (contract over n)
  pass 2: J  = r0 @ W0         (contract over h)
W0 is sharded along n over 8 cores (16K cols/core/pass).

v2 vs baseline (173.9us):
  * W0 slabs are cast to fp16 on the host: halves the HBM traffic
    (4.2MB/core/pass) and makes each matmul single-pass (fp32 matmuls
    are double-pumped LOW/HIGH on TRN2, doubling LDWEIGHTS+MATMUL).
  * pass 1 swaps matmul operands: the tiny x_norm column is the
    stationary (LDWEIGHTS of 1 column ~ 1 cycle) and the W0T chunk is
    the moving operand (128 cols/chunk). The baseline loaded a fresh
    128x128 stationary per chunk: 2x128 LDW columns at 1.2GHz = 427ns
    per chunk, i.e. a 55us serial PE chain - the measured bottleneck.
  * all small inputs packed into ONE DMA per launch (issue cost on the
    sync queue was ~0.64us per dma_start; baseline had 12-17 of them
    gating the first matmul at t=20us).
  * backward chain uses lhsT=natural weights to produce column vectors
    directly (baseline did row-matmul + transpose-matmul pairs).
Still two launches: the 8 partial V0 columns must be summed across
cores before the tanh chain; shuttling 512B/core through the host
between launches is far cheaper than an on-device AllReduce here.
"""

import os
import sys

import numpy as np

sys.path.insert(0, "/opt/trn_rl_repo")

import concourse.tile as tile  # noqa: E402
from concourse import bacc, mybir  # noqa: E402
from concourse import bass_utils  # noqa: E402

N_STATE = 131072
H = 128
N_CORES = 8
N_LOC = N_STATE // N_CORES  # 16384
C = N_LOC // 128  # 128 chunks of 128 per core
# slab tile widths (cols): sized so smalls + tiles <= 8 HWDGE sem lanes.
# Big tiles FIRST: the last tile's matmul burst runs after the final DMA
# byte lands, so keep the tail tiles small.
TILES_A = [4096] + [2048] * 6          # 7 tiles + 1 small = 8 DMAs
TILES_B = [4096] * 2 + [2048] * 4      # 6 tiles + 2 smalls = 8 DMAs

FP = mybir.dt.float32
FH = mybir.dt.float16
AOT = mybir.AluOpType
ACT = mybir.ActivationFunctionType

_CACHE = {}


def _build_a():
    """Launch A: fp16 transposed slab; V0 partial = x_norm @ W0T."""
    nc = bacc.Bacc("TRN2", target_bir_lowering=False, debug=False,
                   num_devices=N_CORES)

    w0t_d = nc.dram_tensor("w0t", [128, N_LOC], FH, kind="ExternalInput").ap()
    # packed: cols 0:128 xsT | 128:256 xmaxT | 256:384 xminT  (all [k, c])
    sm_d = nc.dram_tensor("sm", [128, 384], FP, kind="ExternalInput").ap()
    outp_d = nc.dram_tensor("out_p", [1, H], FP, kind="ExternalOutput").ap()

    with tile.TileContext(nc) as tc:
        with tc.tile_pool(name="w0", bufs=1) as w0p, \
             tc.tile_pool(name="small", bufs=1) as sp, \
             tc.tile_pool(name="psl", bufs=1, space="PSUM") as plp:

            sm = sp.tile([128, 384], FP)
            nc.sync.dma_start(sm[:], sm_d[:])
            w0tiles = []
            off = 0
            for t, w in enumerate(TILES_A):
                w0tile = w0p.tile([128, w], FH, tag=f"w0tile{t}")
                eng = nc.scalar if t % 2 == 0 else nc.sync
                eng.dma_start(w0tile[:], w0t_d[:, off:off + w])
                w0tiles.append((w0tile, off, w))
                off += w

            xsT = sm[:, 0:128]
            xmaxT = sm[:, 128:256]
            xminT = sm[:, 256:384]
            # x_norm = (2*state - (max+min)) / (max-min), in [k, c] layout
            xrw = sp.tile([128, C], FP)
            nc.vector.tensor_sub(xrw[:], xmaxT, xminT)
            invw = sp.tile([128, C], FP)
            nc.vector.reciprocal(invw[:], xrw[:])
            c2 = sp.tile([128, C], FP)
            nc.vector.tensor_add(c2[:], xmaxT, xminT)
            s2 = sp.tile([128, C], FP)
            nc.vector.tensor_scalar_mul(s2[:], xsT, 2.0)
            t2 = sp.tile([128, C], FP)
            nc.vector.tensor_sub(t2[:], s2[:], c2[:])
            xn16 = sp.tile([128, C], FH)
            nc.vector.tensor_mul(xn16[:], t2[:], invw[:])

            # V0[1, h] += xn_c.T @ W0T_chunk_c  -- stationary is 1 column
            v0ps = plp.tile([1, H], FP)
            for w0tile, off, w in w0tiles:
                for cc in range(w // 128):
                    c = off // 128 + cc
                    nc.tensor.matmul(
                        v0ps[:],
                        xn16[:, c:c + 1],
                        w0tile[:, cc * 128:(cc + 1) * 128],
                        start=(c == 0),
                        stop=(c == C - 1),
                    )

            v0sb = sp.tile([1, H], FP)
            nc.vector.tensor_copy(v0sb[:], v0ps[:])
            nc.sync.dma_start(outp_d[:], v0sb[:])

    nc.compile()
    return nc


def _build_b():
    """Launch B: fp16 natural slab; reduce partials + chain + J = r0 @ W0."""
    nc = bacc.Bacc("TRN2", target_bir_lowering=False, debug=False,
                   num_devices=N_CORES)

    w0n_d = nc.dram_tensor("w0n", [H, N_LOC], FH, kind="ExternalInput").ap()
    # sm0 [128, 16]: cols 0:8 bcols (b0,b1,b2,b3,woutT,bout@[0,5]) | 8:16 parts
    sm0_d = nc.dram_tensor("sm0", [128, 16], FP, kind="ExternalInput").ap()
    # sm1 [128, 1024]: w1t | w2t | w3t | w1n | w2n | w3n | xmaxT | xminT
    sm1_d = nc.dram_tensor("sm1", [128, 1024], FP, kind="ExternalInput").ap()

    outj_d = nc.dram_tensor("out_j", [128, C], FP, kind="ExternalOutput").ap()
    outv_d = nc.dram_tensor("out_v", [1, 1], FP, kind="ExternalOutput").ap()

    with tile.TileContext(nc) as tc:
        with tc.tile_pool(name="w0", bufs=1) as w0p, \
             tc.tile_pool(name="small", bufs=1) as sp, \
             tc.tile_pool(name="ps", bufs=2, space="PSUM") as pp, \
             tc.tile_pool(name="psj", bufs=1, space="PSUM") as pjp:

            sm0 = sp.tile([128, 16], FP)
            nc.sync.dma_start(sm0[:], sm0_d[:])
            sm1 = sp.tile([128, 1024], FP)
            nc.scalar.dma_start(sm1[:], sm1_d[:])
            w0tiles = []
            off = 0
            for t, w in enumerate(TILES_B):
                w0tile = w0p.tile([128, w], FH, tag=f"w0tile{t}")
                eng = nc.sync if t % 2 == 0 else nc.scalar
                eng.dma_start(w0tile[:], w0n_d[:, off:off + w])
                w0tiles.append((w0tile, off, w))
                off += w

            w1t = sm1[:, 0:128]
            w2t = sm1[:, 128:256]
            w3t = sm1[:, 256:384]
            w1n = sm1[:, 384:512]
            w2n = sm1[:, 512:640]
            w3n = sm1[:, 640:768]
            bcols = sm0[:, 0:8]
            parts = sm0[:, 8:16]
            xmaxT = sm1[:, 768:896]
            xminT = sm1[:, 896:1024]

            # 1/(xmax-xmin) in [k, c]; the missing *2 is folded into d0.
            xrT = sp.tile([128, C], FP)
            nc.vector.tensor_sub(xrT[:], xmaxT, xminT)
            invT = sp.tile([128, C], FP)
            nc.vector.reciprocal(invT[:], xrT[:])

            # ---- forward chain (vectors as [128, 1] columns) ----
            v0c = sp.tile([H, 1], FP)
            nc.vector.tensor_reduce(v0c[:], parts, mybir.AxisListType.X, AOT.add)

            v1c = sp.tile([H, 1], FP)
            nc.scalar.activation(v1c[:], v0c[:], ACT.Tanh, bias=bcols[:, 0:1])
            d0c = sp.tile([H, 1], FP)  # holds 2*(1 - v1^2)
            nc.vector.tensor_mul(d0c[:], v1c[:], v1c[:])
            nc.vector.tensor_scalar(d0c[:], d0c[:], -2.0, 2.0, AOT.mult, AOT.add)

            a1ps = pp.tile([H, 1], FP, tag="chain_ps")
            nc.tensor.matmul(a1ps[:], w1t, v1c[:], start=True, stop=True)
            v2c = sp.tile([H, 1], FP)
            nc.scalar.activation(v2c[:], a1ps[:], ACT.Tanh, bias=bcols[:, 1:2])
            d1c = sp.tile([H, 1], FP)
            nc.vector.tensor_mul(d1c[:], v2c[:], v2c[:])
            nc.vector.tensor_scalar(d1c[:], d1c[:], -1.0, 1.0, AOT.mult, AOT.add)

            a2ps = pp.tile([H, 1], FP, tag="chain_ps")
            nc.tensor.matmul(a2ps[:], w2t, v2c[:], start=True, stop=True)
            v3c = sp.tile([H, 1], FP)
            nc.scalar.activation(v3c[:], a2ps[:], ACT.Tanh, bias=bcols[:, 2:3])
            d2c = sp.tile([H, 1], FP)
            nc.vector.tensor_mul(d2c[:], v3c[:], v3c[:])
            nc.vector.tensor_scalar(d2c[:], d2c[:], -1.0, 1.0, AOT.mult, AOT.add)

            a3ps = pp.tile([H, 1], FP, tag="chain_ps")
            nc.tensor.matmul(a3ps[:], w3t, v3c[:], start=True, stop=True)
            a3c = sp.tile([H, 1], FP)
            nc.scalar.activation(a3c[:], a3ps[:], ACT.Identity, bias=bcols[:, 3:4])

            voutps = pp.tile([1, 1], FP, tag="chain_ps")
            nc.tensor.matmul(voutps[:], bcols[:, 4:5], a3c[:], start=True, stop=True)
            voutsb = sp.tile([1, 1], FP)
            nc.scalar.activation(voutsb[:], voutps[:], ACT.Identity,
                                 bias=bcols[0:1, 5:6])
            nc.sync.dma_start(outv_d[:], voutsb[:])

            # ---- backward chain, all in column form ----
            # r3 = (Wout @ W3).T = W3.T @ woutT: lhsT = W3 natural.
            r3ps = pp.tile([H, 1], FP, tag="chain_ps")
            nc.tensor.matmul(r3ps[:], w3n, bcols[:, 4:5], start=True, stop=True)
            q2c = sp.tile([H, 1], FP)
            nc.vector.tensor_mul(q2c[:], r3ps[:], d2c[:])

            r2ps = pp.tile([H, 1], FP, tag="chain_ps")
            nc.tensor.matmul(r2ps[:], w2n, q2c[:], start=True, stop=True)
            q1c = sp.tile([H, 1], FP)
            nc.vector.tensor_mul(q1c[:], r2ps[:], d1c[:])

            r1ps = pp.tile([H, 1], FP, tag="chain_ps")
            nc.tensor.matmul(r1ps[:], w1n, q1c[:], start=True, stop=True)
            r0c = sp.tile([H, 1], FP)
            nc.vector.tensor_mul(r0c[:], r1ps[:], d0c[:])
            r016 = sp.tile([H, 1], FH)
            nc.vector.tensor_copy(r016[:], r0c[:])

            # ---- pass 2: J[k, c] = W0_chunk_c.T @ r0 ----
            # split in halves so the first half's scale+store overlaps the
            # second half's matmuls (hides the output-DMA receipt latency)
            jtA = pjp.tile([128, C // 2], FP)
            jtB = pjp.tile([128, C // 2], FP)
            for w0tile, off, w in w0tiles:
                for cc in range(w // 128):
                    c = off // 128 + cc
                    dst = jtA[:, c:c + 1] if c < C // 2 else jtB[:, c - C // 2:c - C // 2 + 1]
                    nc.tensor.matmul(
                        dst,
                        w0tile[:, cc * 128:(cc + 1) * 128],
                        r016[:],
                        start=True,
                        stop=True,
                    )
                if off + w == N_LOC // 2:
                    jtsA = sp.tile([128, C // 2], FP)
                    nc.vector.tensor_mul(jtsA[:], jtA[:], invT[:, 0:C // 2])
                    nc.sync.dma_start(outj_d[:, 0:C // 2], jtsA[:])
                elif off + w == N_LOC - 2048:
                    # flush chunks 64..111 while the last tile's matmuls run
                    jtsB1 = sp.tile([128, 48], FP)
                    nc.vector.tensor_mul(jtsB1[:], jtB[:, 0:48], invT[:, 64:112])
                    nc.scalar.dma_start(outj_d[:, 64:112], jtsB1[:])

            jtsB2 = sp.tile([128, 16], FP)
            nc.vector.tensor_mul(jtsB2[:], jtB[:, 48:64], invT[:, 112:128])
            nc.scalar.dma_start(outj_d[:, 112:128], jtsB2[:])

    nc.compile()
    return nc


def _get_kernels():
    if "nc_a" not in _CACHE:
        _CACHE["nc_a"] = _build_a()
        _CACHE["nc_b"] = _build_b()
    return _CACHE["nc_a"], _CACHE["nc_b"]


def kernel(**inputs):
    nc_a, nc_b = _get_kernels()
    f = np.float32

    state = np.asarray(inputs["state"], f).reshape(1, N_STATE)
    x_max = np.asarray(inputs["x_max"], f).reshape(N_STATE)
    x_min = np.asarray(inputs["x_min"], f).reshape(N_STATE)
    W0 = np.asarray(inputs["W0"], f)
    W1 = np.asarray(inputs["W1"], f)
    W2 = np.asarray(inputs["W2"], f)
    W3 = np.asarray(inputs["W3"], f)
    Wout = np.asarray(inputs["Wout"], f).reshape(1, H)
    b0 = np.asarray(inputs["b0"], f).reshape(H)
    b1 = np.asarray(inputs["b1"], f).reshape(H)
    b2 = np.asarray(inputs["b2"], f).reshape(H)
    b3 = np.asarray(inputs["b3"], f).reshape(H)
    bout = np.asarray(inputs["bout"], f).reshape(1)

    xmaxT = []
    xminT = []
    in_maps_a = []
    for i in range(N_CORES):
        sl = slice(i * N_LOC, (i + 1) * N_LOC)
        w0t16 = np.ascontiguousarray(
            W0[:, sl].reshape(H, C, 128).transpose(2, 1, 0)
        ).reshape(128, C * H).astype(np.float16)
        xmaxT.append(np.ascontiguousarray(x_max[sl].reshape(C, 128).T))
        xminT.append(np.ascontiguousarray(x_min[sl].reshape(C, 128).T))
        sm = np.empty((128, 384), f)
        sm[:, 0:128] = state[0, sl].reshape(C, 128).T
        sm[:, 128:256] = xmaxT[i]
        sm[:, 256:384] = xminT[i]
        in_maps_a.append({"w0t": w0t16, "sm": sm})

    trace = bool(int(os.environ.get("KERNEL_TRACE", "0")))
    res_a = bass_utils.run_bass_kernel_spmd(
        nc_a, in_maps_a, core_ids=list(range(N_CORES)), trace=trace
    )
    _CACHE["res_a"] = res_a

    # pure gather: the 8 per-core [1, 128] partial rows -> [128, 8] columns
    parts = np.ascontiguousarray(
        np.concatenate(
            [np.asarray(res_a.results[i]["out_p"]).reshape(H, 1)
             for i in range(N_CORES)], axis=1)
    )

    sm0 = np.zeros((128, 16), f)
    sm0[:, 0] = b0
    sm0[:, 1] = b1
    sm0[:, 2] = b2
    sm0[:, 3] = b3
    sm0[:, 4] = Wout[0]
    sm0[0, 5] = bout[0]
    sm0[:, 8:16] = parts
    sm1c = np.empty((128, 1024), f)
    sm1c[:, 0:128] = W1.T
    sm1c[:, 128:256] = W2.T
    sm1c[:, 256:384] = W3.T
    sm1c[:, 384:512] = W1
    sm1c[:, 512:640] = W2
    sm1c[:, 640:768] = W3
    in_maps_b = []
    for i in range(N_CORES):
        sl = slice(i * N_LOC, (i + 1) * N_LOC)
        sm1 = sm1c.copy()
        sm1[:, 768:896] = xmaxT[i]
        sm1[:, 896:1024] = xminT[i]
        in_maps_b.append({
            "w0n": np.ascontiguousarray(W0[:, sl]).astype(np.float16),
            "sm0": sm0,
            "sm1": sm1,
        })

    res_b = bass_utils.run_bass_kernel_spmd(
        nc_b, in_maps_b, core_ids=list(range(N_CORES)), trace=trace
    )
    _CACHE["res_b"] = res_b

    out = np.empty((1, N_STATE + 1), np.float32)
    out[0, 0] = float(np.asarray(res_b.results[0]["out_v"]).reshape(()))
    for i in range(N_CORES):
        jt = np.asarray(res_b.results[i]["out_j"])  # [k, c]
        out[0, 1 + i * N_LOC:1 + (i + 1) * N_LOC] = jt.T.reshape(-1)
    return out


# revision 22
# speedup vs baseline: 1.1589x; 1.1589x over previous
"""Trainium2 Bass kernel for nn_CBF (dense MLP forward + Jacobian row).

Math: HJH = [h, Jh] with h = MLP(x_norm) (scalar) and
  Jh = Wout @ W3 @ D2 @ W2 @ D1 @ W1 @ D0 @ (W0 / x_range)  (1 x n row).
The Jacobian chain collapses to r0 @ W0 / x_range with r0 a 128-vector,
so only two passes over the big W0 (128 x 131072) are needed:
  pass 1: V0 = x_norm @ W0.T   (contract over n)
  pass 2: J  = r0 @ W0         (contract over h)
W0 is sharded along n over 8 cores (16K cols/core/pass).

v2 vs baseline (173.9us):
  * W0 slabs are cast to fp16 on the host: halves the HBM traffic
    (4.2MB/core/pass) and makes each matmul single-pass (fp32 matmuls
    are double-pumped LOW/HIGH on TRN2, doubling LDWEIGHTS+MATMUL).
  * pass 1 swaps matmul operands: the tiny x_norm column is the
    stationary (LDWEIGHTS of 1 column ~ 1 cycle) and the W0T chunk is
    the moving operand (128 cols/chunk). The baseline loaded a fresh
    128x128 stationary per chunk: 2x128 LDW columns at 1.2GHz = 427ns
    per chunk, i.e. a 55us serial PE chain - the measured bottleneck.
  * all small inputs packed into ONE DMA per launch (issue cost on the
    sync queue was ~0.64us per dma_start; baseline had 12-17 of them
    gating the first matmul at t=20us).
  * backward chain uses lhsT=natural weights to produce column vectors
    directly (baseline did row-matmul + transpose-matmul pairs).
Still two launches: the 8 partial V0 columns must be summed across
cores before the tanh chain; shuttling 512B/core through the host
between launches is far cheaper than an on-device AllReduce here.
"""

import os
import sys

import numpy as np

sys.path.insert(0, "/opt/trn_rl_repo")

import concourse.tile as tile  # noqa: E402
from concourse import bacc, mybir  # noqa: E402
from concourse import bass_utils  # noqa: E402

N_STATE = 131072
H = 128
N_CORES = 8
N_LOC = N_STATE // N_CORES  # 16384
C = N_LOC // 128  # 128 chunks of 128 per core
# slab tile widths (cols): sized so smalls + tiles <= 8 HWDGE sem lanes.
# Big tiles FIRST: the last tile's matmul burst runs after the final DMA
# byte lands, so keep the tail tiles small.
TILES_A = [4096, 4096, 2048, 2048, 2048, 1024, 1024]  # 7 tiles + 1 small = 8 DMAs
TILES_B = [4096, 4096, 4096, 2048, 1024, 1024]        # 6 tiles + 2 smalls = 8 DMAs

FP = mybir.dt.float32
FH = mybir.dt.float16
AOT = mybir.AluOpType
ACT = mybir.ActivationFunctionType

_CACHE = {}


def _build_a():
    """Launch A: fp16 transposed slab; V0 partial = x_norm @ W0T."""
    nc = bacc.Bacc("TRN2", target_bir_lowering=False, debug=False,
                   num_devices=N_CORES)

    w0t_d = nc.dram_tensor("w0t", [128, N_LOC], FH, kind="ExternalInput").ap()
    # packed: cols 0:128 xsT | 128:256 xmaxT | 256:384 xminT  (all [k, c])
    sm_d = nc.dram_tensor("sm", [128, 384], FP, kind="ExternalInput").ap()
    outp_d = nc.dram_tensor("out_p", [1, H], FP, kind="ExternalOutput").ap()

    with tile.TileContext(nc) as tc:
        with tc.tile_pool(name="w0", bufs=1) as w0p, \
             tc.tile_pool(name="small", bufs=1) as sp, \
             tc.tile_pool(name="psl", bufs=1, space="PSUM") as plp:

            sm = sp.tile([128, 384], FP)
            nc.sync.dma_start(sm[:], sm_d[:])
            w0tiles = []
            off = 0
            for t, w in enumerate(TILES_A):
                w0tile = w0p.tile([128, w], FH, tag=f"w0tile{t}")
                eng = nc.scalar if t % 2 == 0 else nc.sync
                eng.dma_start(w0tile[:], w0t_d[:, off:off + w])
                w0tiles.append((w0tile, off, w))
                off += w

            xsT = sm[:, 0:128]
            xmaxT = sm[:, 128:256]
            xminT = sm[:, 256:384]
            # x_norm = (2*state - (max+min)) / (max-min), in [k, c] layout
            xrw = sp.tile([128, C], FP)
            nc.vector.tensor_sub(xrw[:], xmaxT, xminT)
            invw = sp.tile([128, C], FP)
            nc.vector.reciprocal(invw[:], xrw[:])
            c2 = sp.tile([128, C], FP)
            nc.vector.tensor_add(c2[:], xmaxT, xminT)
            s2 = sp.tile([128, C], FP)
            nc.vector.tensor_scalar_mul(s2[:], xsT, 2.0)
            t2 = sp.tile([128, C], FP)
            nc.vector.tensor_sub(t2[:], s2[:], c2[:])
            xn16 = sp.tile([128, C], FH)
            nc.vector.tensor_mul(xn16[:], t2[:], invw[:])

            # V0[1, h] += xn_c.T @ W0T_chunk_c  -- stationary is 1 column
            v0ps = plp.tile([1, H], FP)
            for w0tile, off, w in w0tiles:
                for cc in range(w // 128):
                    c = off // 128 + cc
                    nc.tensor.matmul(
                        v0ps[:],
                        xn16[:, c:c + 1],
                        w0tile[:, cc * 128:(cc + 1) * 128],
                        start=(c == 0),
                        stop=(c == C - 1),
                    )

            v0sb = sp.tile([1, H], FP)
            nc.vector.tensor_copy(v0sb[:], v0ps[:])
            nc.sync.dma_start(outp_d[:], v0sb[:])

    nc.compile()
    return nc


def _build_b():
    """Launch B: fp16 natural slab; reduce partials + chain + J = r0 @ W0."""
    nc = bacc.Bacc("TRN2", target_bir_lowering=False, debug=False,
                   num_devices=N_CORES)

    w0n_d = nc.dram_tensor("w0n", [H, N_LOC], FH, kind="ExternalInput").ap()
    # sm0 [128, 16]: cols 0:8 bcols (b0,b1,b2,b3,woutT,bout@[0,5]) | 8:16 parts
    sm0_d = nc.dram_tensor("sm0", [128, 16], FP, kind="ExternalInput").ap()
    # sm1 [128, 1024]: w1t | w2t | w3t | w1n | w2n | w3n | xmaxT | xminT
    sm1_d = nc.dram_tensor("sm1", [128, 1024], FP, kind="ExternalInput").ap()

    outj_d = nc.dram_tensor("out_j", [128, C], FP, kind="ExternalOutput").ap()
    outv_d = nc.dram_tensor("out_v", [1, 1], FP, kind="ExternalOutput").ap()

    with tile.TileContext(nc) as tc:
        with tc.tile_pool(name="w0", bufs=1) as w0p, \
             tc.tile_pool(name="small", bufs=1) as sp, \
             tc.tile_pool(name="ps", bufs=2, space="PSUM") as pp, \
             tc.tile_pool(name="psj", bufs=1, space="PSUM") as pjp:

            sm0 = sp.tile([128, 16], FP)
            nc.sync.dma_start(sm0[:], sm0_d[:])
            sm1 = sp.tile([128, 1024], FP)
            nc.scalar.dma_start(sm1[:], sm1_d[:])
            w0tiles = []
            off = 0
            for t, w in enumerate(TILES_B):
                w0tile = w0p.tile([128, w], FH, tag=f"w0tile{t}")
                eng = nc.sync if t % 2 == 0 else nc.scalar
                eng.dma_start(w0tile[:], w0n_d[:, off:off + w])
                w0tiles.append((w0tile, off, w))
                off += w

            w1t = sm1[:, 0:128]
            w2t = sm1[:, 128:256]
            w3t = sm1[:, 256:384]
            w1n = sm1[:, 384:512]
            w2n = sm1[:, 512:640]
            w3n = sm1[:, 640:768]
            bcols = sm0[:, 0:8]
            parts = sm0[:, 8:16]
            xmaxT = sm1[:, 768:896]
            xminT = sm1[:, 896:1024]

            # 1/(xmax-xmin) in [k, c]; the missing *2 is folded into d0.
            xrT = sp.tile([128, C], FP)
            nc.vector.tensor_sub(xrT[:], xmaxT, xminT)
            invT = sp.tile([128, C], FP)
            nc.vector.reciprocal(invT[:], xrT[:])

            # ---- forward chain (vectors as [128, 1] columns) ----
            v0c = sp.tile([H, 1], FP)
            nc.vector.tensor_reduce(v0c[:], parts, mybir.AxisListType.X, AOT.add)

            v1c = sp.tile([H, 1], FP)
            nc.scalar.activation(v1c[:], v0c[:], ACT.Tanh, bias=bcols[:, 0:1])
            d0c = sp.tile([H, 1], FP)  # holds 2*(1 - v1^2)
            nc.vector.tensor_mul(d0c[:], v1c[:], v1c[:])
            nc.vector.tensor_scalar(d0c[:], d0c[:], -2.0, 2.0, AOT.mult, AOT.add)

            a1ps = pp.tile([H, 1], FP, tag="chain_ps")
            nc.tensor.matmul(a1ps[:], w1t, v1c[:], start=True, stop=True)
            v2c = sp.tile([H, 1], FP)
            nc.scalar.activation(v2c[:], a1ps[:], ACT.Tanh, bias=bcols[:, 1:2])
            d1c = sp.tile([H, 1], FP)
            nc.vector.tensor_mul(d1c[:], v2c[:], v2c[:])
            nc.vector.tensor_scalar(d1c[:], d1c[:], -1.0, 1.0, AOT.mult, AOT.add)

            a2ps = pp.tile([H, 1], FP, tag="chain_ps")
            nc.tensor.matmul(a2ps[:], w2t, v2c[:], start=True, stop=True)
            v3c = sp.tile([H, 1], FP)
            nc.scalar.activation(v3c[:], a2ps[:], ACT.Tanh, bias=bcols[:, 2:3])
            d2c = sp.tile([H, 1], FP)
            nc.vector.tensor_mul(d2c[:], v3c[:], v3c[:])
            nc.vector.tensor_scalar(d2c[:], d2c[:], -1.0, 1.0, AOT.mult, AOT.add)

            a3ps = pp.tile([H, 1], FP, tag="chain_ps")
            nc.tensor.matmul(a3ps[:], w3t, v3c[:], start=True, stop=True)
            a3c = sp.tile([H, 1], FP)
            nc.scalar.activation(a3c[:], a3ps[:], ACT.Identity, bias=bcols[:, 3:4])

            voutps = pp.tile([1, 1], FP, tag="chain_ps")
            nc.tensor.matmul(voutps[:], bcols[:, 4:5], a3c[:], start=True, stop=True)
            voutsb = sp.tile([1, 1], FP)
            nc.scalar.activation(voutsb[:], voutps[:], ACT.Identity,
                                 bias=bcols[0:1, 5:6])
            nc.sync.dma_start(outv_d[:], voutsb[:])

            # ---- backward chain, all in column form ----
            # r3 = (Wout @ W3).T = W3.T @ woutT: lhsT = W3 natural.
            r3ps = pp.tile([H, 1], FP, tag="chain_ps")
            nc.tensor.matmul(r3ps[:], w3n, bcols[:, 4:5], start=True, stop=True)
            q2c = sp.tile([H, 1], FP)
            nc.vector.tensor_mul(q2c[:], r3ps[:], d2c[:])

            r2ps = pp.tile([H, 1], FP, tag="chain_ps")
            nc.tensor.matmul(r2ps[:], w2n, q2c[:], start=True, stop=True)
            q1c = sp.tile([H, 1], FP)
            nc.vector.tensor_mul(q1c[:], r2ps[:], d1c[:])

            r1ps = pp.tile([H, 1], FP, tag="chain_ps")
            nc.tensor.matmul(r1ps[:], w1n, q1c[:], start=True, stop=True)
            r0c = sp.tile([H, 1], FP)
            nc.vector.tensor_mul(r0c[:], r1ps[:], d0c[:])
            r016 = sp.tile([H, 1], FH)
            nc.vector.tensor_copy(r016[:], r0c[:])

            # ---- pass 2: J[k, c] = W0_chunk_c.T @ r0 ----
            # split in halves so the first half's scale+store overlaps the
            # second half's matmuls (hides the output-DMA receipt latency)
            jtA = pjp.tile([128, C // 2], FP)
            jtB = pjp.tile([128, C // 2], FP)
            for w0tile, off, w in w0tiles:
                for cc in range(w // 128):
                    c = off // 128 + cc
                    dst = jtA[:, c:c + 1] if c < C // 2 else jtB[:, c - C // 2:c - C // 2 + 1]
                    nc.tensor.matmul(
                        dst,
                        w0tile[:, cc * 128:(cc + 1) * 128],
                        r016[:],
                        start=True,
                        stop=True,
                    )
                if off + w == N_LOC // 2:
                    jtsA = sp.tile([128, C // 2], FP)
                    nc.vector.tensor_mul(jtsA[:], jtA[:], invT[:, 0:C // 2])
                    nc.sync.dma_start(outj_d[:, 0:C // 2], jtsA[:])
                elif off + w == N_LOC - 2048:
                    # flush chunks 64..111 while the last tile's matmuls run
                    jtsB1 = sp.tile([128, 48], FP)
                    nc.vector.tensor_mul(jtsB1[:], jtB[:, 0:48], invT[:, 64:112])
                    nc.scalar.dma_start(outj_d[:, 64:112], jtsB1[:])

            jtsB2 = sp.tile([128, 16], FP)
            nc.vector.tensor_mul(jtsB2[:], jtB[:, 48:64], invT[:, 112:128])
            nc.scalar.dma_start(outj_d[:, 112:128], jtsB2[:])

    nc.compile()
    return nc


def _get_kernels():
    if "nc_a" not in _CACHE:
        _CACHE["nc_a"] = _build_a()
        _CACHE["nc_b"] = _build_b()
    return _CACHE["nc_a"], _CACHE["nc_b"]


def kernel(**inputs):
    nc_a, nc_b = _get_kernels()
    f = np.float32

    state = np.asarray(inputs["state"], f).reshape(1, N_STATE)
    x_max = np.asarray(inputs["x_max"], f).reshape(N_STATE)
    x_min = np.asarray(inputs["x_min"], f).reshape(N_STATE)
    W0 = np.asarray(inputs["W0"], f)
    W1 = np.asarray(inputs["W1"], f)
    W2 = np.asarray(inputs["W2"], f)
    W3 = np.asarray(inputs["W3"], f)
    Wout = np.asarray(inputs["Wout"], f).reshape(1, H)
    b0 = np.asarray(inputs["b0"], f).reshape(H)
    b1 = np.asarray(inputs["b1"], f).reshape(H)
    b2 = np.asarray(inputs["b2"], f).reshape(H)
    b3 = np.asarray(inputs["b3"], f).reshape(H)
    bout = np.asarray(inputs["bout"], f).reshape(1)

    xmaxT = []
    xminT = []
    in_maps_a = []
    for i in range(N_CORES):
        sl = slice(i * N_LOC, (i + 1) * N_LOC)
        w0t16 = np.ascontiguousarray(
            W0[:, sl].reshape(H, C, 128).transpose(2, 1, 0)
        ).reshape(128, C * H).astype(np.float16)
        xmaxT.append(np.ascontiguousarray(x_max[sl].reshape(C, 128).T))
        xminT.append(np.ascontiguousarray(x_min[sl].reshape(C, 128).T))
        sm = np.empty((128, 384), f)
        sm[:, 0:128] = state[0, sl].reshape(C, 128).T
        sm[:, 128:256] = xmaxT[i]
        sm[:, 256:384] = xminT[i]
        in_maps_a.append({"w0t": w0t16, "sm": sm})

    trace = bool(int(os.environ.get("KERNEL_TRACE", "0")))
    res_a = bass_utils.run_bass_kernel_spmd(
        nc_a, in_maps_a, core_ids=list(range(N_CORES)), trace=trace
    )
    _CACHE["res_a"] = res_a

    # pure gather: the 8 per-core [1, 128] partial rows -> [128, 8] columns
    parts = np.ascontiguousarray(
        np.concatenate(
            [np.asarray(res_a.results[i]["out_p"]).reshape(H, 1)
             for i in range(N_CORES)], axis=1)
    )

    sm0 = np.zeros((128, 16), f)
    sm0[:, 0] = b0
    sm0[:, 1] = b1
    sm0[:, 2] = b2
    sm0[:, 3] = b3
    sm0[:, 4] = Wout[0]
    sm0[0, 5] = bout[0]
    sm0[:, 8:16] = parts
    sm1c = np.empty((128, 1024), f)
    sm1c[:, 0:128] = W1.T
    sm1c[:, 128:256] = W2.T
    sm1c[:, 256:384] = W3.T
    sm1c[:, 384:512] = W1
    sm1c[:, 512:640] = W2
    sm1c[:, 640:768] = W3
    in_maps_b = []
    for i in range(N_CORES):
        sl = slice(i * N_LOC, (i + 1) * N_LOC)
        sm1 = sm1c.copy()
        sm1[:, 768:896] = xmaxT[i]
        sm1[:, 896:1024] = xminT[i]
        in_maps_b.append({
            "w0n": np.ascontiguousarray(W0[:, sl]).astype(np.float16),
            "sm0": sm0,
            "sm1": sm1,
        })

    res_b = bass_utils.run_bass_kernel_spmd(
        nc_b, in_maps_b, core_ids=list(range(N_CORES)), trace=trace
    )
    _CACHE["res_b"] = res_b

    out = np.empty((1, N_STATE + 1), np.float32)
    out[0, 0] = float(np.asarray(res_b.results[0]["out_v"]).reshape(()))
    for i in range(N_CORES):
        jt = np.asarray(res_b.results[i]["out_j"])  # [k, c]
        out[0, 1 + i * N_LOC:1 + (i + 1) * N_LOC] = jt.T.reshape(-1)
    return out
